# revision 15
# baseline (speedup 1.0000x reference)
"""Trainium2 Bass kernel for single-head causal attention.

Problem: x[4,2048,1024] f32; Wq/Wk/Wv [1024,1024] (torch Linear layout, y = x@W.T).
  q,k,v = x@W.T ; scores = q@k.T (causal masked, scaled 1/sqrt(1024)) ;
  out = softmax(scores)@v.

Weight folding (the key algebraic move): scores = xq (Wq^T Wk) xk^T, so with
M := Wq^T Wk precomputed on the host, the K projection disappears entirely --
x^T itself is the key matrix. Likewise out = w @ x @ Wv^T, so the V projection
collapses to a small per-slot (w.x) @ Wv^T postmultiply. Device matmul work
drops from q/k/v projections + attention to: one folded q-projection
(xq @ M), QK against raw x^T, (w @ x), and (wx) @ Wv^T.

Sharding: 2 cores per batch (4 batches x 2 = 8 cores). Within a batch the 16
query blocks of 128 rows are split zig-zag so both cores get identical work
AND an identical program structure: core h=0 gets blocks [0,15,2,13,4,11,6,9],
h=1 gets [1,14,3,12,5,10,7,8]. Both orderings give causal key extents of
[1,8,2,7,3,6,4,5] chunks of 256 keys per slot, so a single SPMD program serves
all 8 cores; per-core data (x slices/transposes, gathered query rows, causal
masks, folded M) is prepared on the host.

Per-core pipeline (bf16 matmul inputs, fp32 PSUM accumulation):
  1. qMT = (xq @ M)^T via PE (the only projection, 1024 rows).
  2. QK phase (slots in descending causal length): scores chunks vs resident
     x^T, host mask on the causal edge, row-max -> exp((s-max)/32) on ACT with
     accumulated row sum; all softmax chains hide under later slots' matmuls.
  3. PV pass A: per slot, PE-transpose of weight blocks + (w @ x) accumulation
     over key blocks, with the (wx) transposes of the previous slot
     interleaved so PE never waits on ACT copies.
  4. PV pass B: per slot, (wx)^T @ Wv^T accumulated over d, 1/sum scaling
     fused into the PSUM->SBUF out copy, DMA out.
"""

from contextlib import ExitStack

import ml_dtypes
import numpy as np

import concourse.mybir as mybir
import concourse.tile as tile
from concourse import bacc
from concourse.bass_utils import run_bass_kernel_spmd
from concourse.masks import make_identity

B, S, D, E = 4, 2048, 1024, 1024
P = 128
N_CORES = 8
DT = D // P          # 8 d-tiles (contraction)
SQ = S // 2          # 1024 query rows per core
KCH = 256            # causal-length granularity (key chunk)
NSLOT = SQ // P      # 8 query slots per core

QCH = [128, 384, 512]             # xqT chunking (small first for startup)
assert sum(QCH) == SQ

# zig-zag query-block assignment: both cores' slots have identical causal
# chunk counts CJ, so one SPMD program serves all cores.
QBLOCKS = [[0, 15, 2, 13, 4, 11, 6, 9], [1, 14, 3, 12, 5, 10, 7, 8]]
CJ = [(b + 1 + 1) // 2 for b in QBLOCKS[0]]  # [1,8,2,7,3,6,4,5]
assert CJ == [(b + 1 + 1) // 2 for b in QBLOCKS[1]]
SLOT_ORDER = sorted(range(NSLOT), key=lambda j: -CJ[j])  # longest first

F32 = mybir.dt.float32
BF16 = mybir.dt.bfloat16
AX = mybir.AxisListType.X
EXP = mybir.ActivationFunctionType.Exp
INV_SQRT_DK = 1.0 / 32.0
MASK_VAL = -1.0e9
WPIECES = [(0, 1), (1, 2), (2, 4), (4, 8)]  # M DMA split over out-tiles


def build_kernel():
    nc = bacc.Bacc(
        "TRN2",
        target_bir_lowering=False,
        debug=False,
        num_devices=N_CORES,
        dynamic_dma_scratch_size=64,
    )
    xT_d = nc.dram_tensor("xT", [P, DT, S], BF16, kind="ExternalInput")
    xn_d = nc.dram_tensor("xn", [P, S // P, D], BF16, kind="ExternalInput")
    xqT_d = nc.dram_tensor("xqT", [P, DT, SQ], BF16, kind="ExternalInput")
    m_d = nc.dram_tensor("MT", [P, DT, DT, P], BF16, kind="ExternalInput")
    wv_d = nc.dram_tensor("WvT", [P, DT, E], BF16, kind="ExternalInput")
    msk_d = nc.dram_tensor("masks", [P, NSLOT, KCH], F32, kind="ExternalInput")
    out_d = nc.dram_tensor("out", [SQ, E], F32, kind="ExternalOutput")

    with tile.TileContext(nc) as tc, ExitStack() as ctx:
        # persistent tensors (right side)
        kqv = ctx.enter_context(tc.tile_pool(name="kqv", bufs=1, side="right"))
        xT = kqv.tile([P, DT, S], BF16, tag="xT")        # keys: x^T
        xn = kqv.tile([P, S // P, D], BF16, tag="xn")    # x natural [kb, d]
        qMT = kqv.tile([P, DT, SQ], BF16, tag="qMT")     # (xq M)^T
        wvT = kqv.tile([P, DT, E], BF16, tag="wvT")
        msk = kqv.tile([P, NSLOT, KCH], F32, tag="msk")

        # ---------------- folded q projection ----------------
        with (
            tc.tile_pool(name="wpool", bufs=1) as wpool,
            tc.tile_pool(name="xpool", bufs=3) as xpool,
            tc.tile_pool(name="pps", bufs=6, space="PSUM") as pps,
        ):
            m_sb = wpool.tile([P, DT, DT, P], BF16, tag="M", name="m_sb")
            lo, hi = WPIECES[0]
            nc.sync.dma_start(m_sb[:, lo:hi], m_d[:, lo:hi])
            xqc = []
            t0 = 0
            for ci, csz in enumerate(QCH):
                xc = xpool.tile([P, DT, 512], BF16, tag="x", name="xc")
                nc.sync.dma_start(xc[:, :, 0:csz], xqT_d[:, :, t0 : t0 + csz])
                xqc.append(xc)
                t0 += csz
                if ci == 0:
                    for lo, hi in WPIECES[1:]:
                        nc.sync.dma_start(m_sb[:, lo:hi], m_d[:, lo:hi])
            # bulk streaming inputs, ordered by first use in the attention
            # phases: xT (QK), masks (first causal edge ~45us), xn (pass A),
            # WvT (pass B)
            nc.sync.dma_start(xT[:], xT_d[:])
            nc.sync.dma_start(msk[:], msk_d[:])
            nc.sync.dma_start(xn[:], xn_d[:])
            for lo, hi in WPIECES:
                nc.sync.dma_start(wvT[:, lo:hi], wv_d[:, lo:hi])

            t0 = 0
            for ci, csz in enumerate(QCH):
                xc = xqc[ci]
                for j_t in range(DT):
                    ps = pps.tile([P, 512], F32, tag="ps", name="ps")
                    for d in range(DT):
                        nc.tensor.matmul(
                            ps[:, 0:csz],
                            lhsT=m_sb[:, j_t, d, :],
                            rhs=xc[:, d, 0:csz],
                            start=(d == 0),
                            stop=(d == DT - 1),
                        )
                    nc.scalar.copy(qMT[:, j_t, t0 : t0 + csz], ps[:, 0:csz])
                t0 += csz

        # ---------------- attention ----------------
        with (
            tc.tile_pool(name="apool", bufs=2) as apool,
            tc.tile_pool(name="wtpool", bufs=4) as wtpool,
            tc.tile_pool(name="wxpool", bufs=2) as wxpool,
            tc.tile_pool(name="wxtpool", bufs=NSLOT) as wxtpool,
            tc.tile_pool(name="stpool", bufs=NSLOT, side="right") as stpool,
            tc.tile_pool(name="c1pool", bufs=1) as c1pool,
        ):
            ident = c1pool.tile([P, P], BF16, tag="ident")
            make_identity(nc, ident[:])

            def emit_scores(j):
                """QK (512-wide chunks) + mask + max + exp + sum for slot j."""
                C = CJ[j]
                L = C * KCH
                scores = apool.tile([P, S], F32, tag="scores", name="scores")
                groups = [(g * 512, 512) for g in range(C // 2)]
                if C % 2:
                    groups.append(((C // 2) * 512, 256))
                for k0, ksz in groups:
                    ps = qkps.tile([P, 512], F32, tag="qk", name="qk")
                    for d in range(DT):
                        nc.tensor.matmul(
                            ps[:, 0:ksz],
                            lhsT=qMT[:, d, j * P : (j + 1) * P],
                            rhs=xT[:, d, k0 : k0 + ksz],
                            start=(d == 0),
                            stop=(d == DT - 1),
                        )
                    last = k0 + ksz == L
                    if last and ksz == 512:
                        nc.vector.tensor_copy(
                            scores[:, k0 : k0 + 256], ps[:, 0:256]
                        )
                        nc.vector.tensor_add(
                            scores[:, k0 + 256 : k0 + 512],
                            ps[:, 256:512],
                            msk[:, j, :],
                        )
                    elif last:
                        nc.vector.tensor_add(
                            scores[:, k0 : k0 + 256], ps[:, 0:256], msk[:, j, :]
                        )
                    else:
                        nc.vector.tensor_copy(
                            scores[:, k0 : k0 + ksz], ps[:, 0:ksz]
                        )

                st = stpool.tile([P, 4], F32, tag="st", name="st")
                nc.vector.tensor_reduce(
                    st[:, 0:1], scores[:, 0:L], axis=AX, op=mybir.AluOpType.max
                )
                nc.scalar.mul(st[:, 1:2], st[:, 0:1], -INV_SQRT_DK)
                wts = apool.tile(
                    [P, S], BF16, tag="wts", name="wts", bufs=NSLOT
                )
                nc.scalar.activation(
                    wts[:, 0:L],
                    scores[:, 0:L],
                    EXP,
                    bias=st[:, 1:2],
                    scale=INV_SQRT_DK,
                    accum_out=st[:, 2:3],
                )
                nc.vector.reciprocal(st[:, 3:4], st[:, 2:3])
                return wts, st

            with tc.tile_pool(name="qkps", bufs=4, space="PSUM") as qkps:
                staged = [(j, *emit_scores(j)) for j in SLOT_ORDER]

            # ---- PV pass A: w transposes + (w @ x); previous slot's (wx)
            # transposes interleave so PE doesn't wait on ACT copies.
            wxT_all = []

            with (
                tc.tile_pool(name="wxps", bufs=4, space="PSUM") as wxps,
                tc.tile_pool(name="trps", bufs=4, space="PSUM") as trps,
            ):

                def make_wx_tr(si):
                    """per-d emitters: transpose (wx)[q,d] -> wxT[d,q]."""
                    wx_sb, _ = wx_staged[si]
                    wxT = wxtpool.tile(
                        [P, DT, P], BF16, tag="wxT", name="wxT"
                    )
                    wxT_all.append(wxT)

                    def emit_one(d):
                        pt = trps.tile([P, P], BF16, tag="tr", name="pt")
                        nc.tensor.transpose(
                            pt[:], wx_sb[:, d * P : (d + 1) * P], ident[:]
                        )
                        nc.vector.tensor_copy(wxT[:, d, :], pt[:])

                    return [emit_one(d) if False else (lambda d=d: emit_one(d))
                            for d in range(DT)]

                wx_staged = []
                pending_tr = []
                for si, (j, wts, st) in enumerate(staged):
                    nkb = CJ[j] * KCH // P
                    # weight transposes (one block lookahead inside the slot)
                    wTq = []

                    def emit_tr(kb, wts=wts):
                        pt = trps.tile([P, P], BF16, tag="tr", name="pt")
                        nc.tensor.transpose(
                            pt[:], wts[:, kb * P : (kb + 1) * P], ident[:]
                        )
                        wT = wtpool.tile([P, P], BF16, tag="wT", name="wT")
                        nc.vector.tensor_copy(wT[:], pt[:])
                        wTq.append(wT)

                    emit_tr(0)
                    if nkb > 1:
                        emit_tr(1)
                    po = [
                        wxps.tile([P, 512], F32, tag="wx", name=f"wx{ec}")
                        for ec in range(2)
                    ]
                    for kb in range(nkb):
                        if kb + 2 < nkb:
                            emit_tr(kb + 2)
                        if pending_tr:
                            pending_tr.pop(0)()
                        for ec in range(2):
                            nc.tensor.matmul(
                                po[ec][:],
                                lhsT=wTq[kb][:],
                                rhs=xn[:, kb, ec * 512 : (ec + 1) * 512],
                                start=(kb == 0),
                                stop=(kb == nkb - 1),
                            )
                    wx_sb = apool.tile(
                        [P, E], BF16, tag="wx", name="wx_sb", bufs=3
                    )
                    for ec in range(2):
                        nc.scalar.copy(
                            wx_sb[:, ec * 512 : (ec + 1) * 512], po[ec][:]
                        )
                    wx_staged.append((wx_sb, st))
                    for fn in pending_tr:
                        fn()
                    pending_tr = make_wx_tr(si)
                for fn in pending_tr:
                    fn()

            # ---- PV pass B: (wx)^T @ Wv^T, scaled by 1/sum, DMA out.
            with tc.tile_pool(name="pvps", bufs=4, space="PSUM") as pvps:
                for si, (j, _, st) in enumerate(staged):
                    wxT = wxT_all[si]
                    po = [
                        pvps.tile([P, 512], F32, tag="pv", name=f"po{ec}")
                        for ec in range(2)
                    ]
                    for d in range(DT):
                        for ec in range(2):
                            nc.tensor.matmul(
                                po[ec][:],
                                lhsT=wxT[:, d, :],
                                rhs=wvT[:, d, ec * 512 : (ec + 1) * 512],
                                start=(d == 0),
                                stop=(d == DT - 1),
                            )
                    ot = apool.tile([P, E], F32, tag="out", name="ot")
                    for ec in range(2):
                        nc.scalar.mul(
                            ot[:, ec * 512 : (ec + 1) * 512],
                            po[ec][:],
                            st[:, 3:4],
                        )
                    nc.sync.dma_start(out_d[j * P : (j + 1) * P, :], ot[:])

    nc.compile()
    return nc


_NC_CACHE = None


def _get_nc():
    global _NC_CACHE
    if _NC_CACHE is None:
        _NC_CACHE = build_kernel()
    return _NC_CACHE


def _pack_inputs(x, Wq, Wk, Wv):
    """Host-side relayout + weight folding."""
    bf = ml_dtypes.bfloat16

    # folded scores matrix: scores = xq @ M @ xk^T with M = Wq^T @ Wk.
    # packed like a torch-Linear weight W_eff = M^T, lhsT[i,j] slices:
    # [p, j_t, i_t, j_local] = M[i_t*128+p, j_t*128+j_local]
    Mt = (Wk.T.astype(np.float64) @ Wq.astype(np.float64)).astype(np.float32)
    mp = np.ascontiguousarray(
        Mt.reshape(DT, P, DT, P).transpose(3, 0, 2, 1).astype(bf)
    )
    # Wv packed d-outer: [p, d, e] = Wv[e, d*128+p] (contiguous rhs slices)
    wvp = np.ascontiguousarray(
        Wv.reshape(E, DT, P).transpose(2, 1, 0).astype(bf)
    )

    # causal masks per slot (identical formula for both cores' block lists)
    def packmask(blocks):
        m = np.zeros((NSLOT, P, KCH), np.float32)
        for j, blk in enumerate(blocks):
            cc = np.arange(KCH)[None, :] + (CJ[j] - 1) * KCH  # key col
            rr = np.arange(P)[:, None] + blk * P              # query row
            m[j] = np.where(cc <= rr, 0.0, MASK_VAL)
        return np.ascontiguousarray(m.transpose(1, 0, 2))     # [P, slot, KCH]

    masks = [packmask(QBLOCKS[0]), packmask(QBLOCKS[1])]

    in_maps = []
    for c in range(N_CORES):
        b, h = divmod(c, 2)
        xb = x[b]  # [S, D]
        xt = np.ascontiguousarray(
            xb.reshape(S, DT, P).transpose(2, 1, 0).astype(bf)
        )
        xnat = np.ascontiguousarray(
            xb.reshape(S // P, P, D).transpose(1, 0, 2).astype(bf)
        )
        rows = np.concatenate(
            [np.arange(blk * P, (blk + 1) * P) for blk in QBLOCKS[h]]
        )
        xq = xb[rows]  # [SQ, D]
        xqt = np.ascontiguousarray(
            xq.reshape(SQ, DT, P).transpose(2, 1, 0).astype(bf)
        )
        in_maps.append(
            {
                "xT": xt,
                "xn": xnat,
                "xqT": xqt,
                "MT": mp,
                "WvT": wvp,
                "masks": masks[h],
            }
        )
    return in_maps


def kernel(x, Wq, Wk, Wv, _spmd_kwargs=None, _results_out=None):
    x = np.asarray(x, dtype=np.float32)
    Wq = np.asarray(Wq, dtype=np.float32)
    Wk = np.asarray(Wk, dtype=np.float32)
    Wv = np.asarray(Wv, dtype=np.float32)
    assert x.shape == (B, S, D)

    nc = _get_nc()
    in_maps = _pack_inputs(x, Wq, Wk, Wv)
    res = run_bass_kernel_spmd(
        nc, in_maps, list(range(N_CORES)), **(_spmd_kwargs or {})
    )
    if _results_out is not None:
        _results_out.append(res)

    out = np.empty((B, S, E), np.float32)
    for c in range(N_CORES):
        b, h = divmod(c, 2)
        o = res.results[c]["out"]
        for j, blk in enumerate(QBLOCKS[h]):
            out[b, blk * P : (blk + 1) * P, :] = o[j * P : (j + 1) * P, :]
    return out


# revision 16
# speedup vs baseline: 1.0070x; 1.0070x over previous
"""Trainium2 Bass kernel for single-head causal attention.

Problem: x[4,2048,1024] f32; Wq/Wk/Wv [1024,1024] (torch Linear layout, y = x@W.T).
  q,k,v = x@W.T ; scores = q@k.T (causal masked, scaled 1/sqrt(1024)) ;
  out = softmax(scores)@v.

Weight folding (the key algebraic move): scores = xq (Wq^T Wk) xk^T, so with
M := Wq^T Wk precomputed on the host, the K projection disappears entirely --
x^T itself is the key matrix. Likewise out = w @ x @ Wv^T, so the V projection
collapses to a small per-slot (w.x) @ Wv^T postmultiply. Device matmul work
drops from q/k/v projections + attention to: one folded q-projection
(xq @ M), QK against raw x^T, (w @ x), and (wx) @ Wv^T.

Sharding: 2 cores per batch (4 batches x 2 = 8 cores). Within a batch the 16
query blocks of 128 rows are split zig-zag so both cores get identical work
AND an identical program structure: core h=0 gets blocks [0,15,2,13,4,11,6,9],
h=1 gets [1,14,3,12,5,10,7,8]. Both orderings give causal key extents of
[1,8,2,7,3,6,4,5] chunks of 256 keys per slot, so a single SPMD program serves
all 8 cores; per-core data (x slices/transposes, gathered query rows, causal
masks, folded M) is prepared on the host.

Per-core pipeline (bf16 matmul inputs, fp32 PSUM accumulation):
  1. qMT = (xq @ M)^T via PE (the only projection, 1024 rows).
  2. QK phase (slots in descending causal length): scores chunks vs resident
     x^T, host mask on the causal edge, row-max -> exp((s-max)/32) on ACT with
     accumulated row sum; all softmax chains hide under later slots' matmuls.
  3. PV pass A: per slot, PE-transpose of weight blocks + (w @ x) accumulation
     over key blocks, with the (wx) transposes of the previous slot
     interleaved so PE never waits on ACT copies.
  4. PV pass B: per slot, (wx)^T @ Wv^T accumulated over d, 1/sum scaling
     fused into the PSUM->SBUF out copy, DMA out.
"""

from contextlib import ExitStack

import ml_dtypes
import numpy as np

import concourse.mybir as mybir
import concourse.tile as tile
from concourse import bacc
from concourse.bass_utils import run_bass_kernel_spmd
from concourse.masks import make_identity

B, S, D, E = 4, 2048, 1024, 1024
P = 128
N_CORES = 8
DT = D // P          # 8 d-tiles (contraction)
SQ = S // 2          # 1024 query rows per core
KCH = 256            # causal-length granularity (key chunk)
NSLOT = SQ // P      # 8 query slots per core

QCH = [256, 256, 512]             # xqT chunking (small first for startup)
assert sum(QCH) == SQ

# zig-zag query-block assignment: both cores' slots have identical causal
# chunk counts CJ, so one SPMD program serves all cores.
QBLOCKS = [[0, 15, 2, 13, 4, 11, 6, 9], [1, 14, 3, 12, 5, 10, 7, 8]]
CJ = [(b + 1 + 1) // 2 for b in QBLOCKS[0]]  # [1,8,2,7,3,6,4,5]
assert CJ == [(b + 1 + 1) // 2 for b in QBLOCKS[1]]
SLOT_ORDER = sorted(range(NSLOT), key=lambda j: -CJ[j])  # longest first

F32 = mybir.dt.float32
BF16 = mybir.dt.bfloat16
AX = mybir.AxisListType.X
EXP = mybir.ActivationFunctionType.Exp
INV_SQRT_DK = 1.0 / 32.0
MASK_VAL = -1.0e9
WPIECES = [(0, 1), (1, 2), (2, 4), (4, 8)]  # M DMA split over out-tiles


def build_kernel():
    nc = bacc.Bacc(
        "TRN2",
        target_bir_lowering=False,
        debug=False,
        num_devices=N_CORES,
        dynamic_dma_scratch_size=64,
    )
    xT_d = nc.dram_tensor("xT", [P, DT, S], BF16, kind="ExternalInput")
    xn_d = nc.dram_tensor("xn", [P, S // P, D], BF16, kind="ExternalInput")
    xqT_d = nc.dram_tensor("xqT", [P, DT, SQ], BF16, kind="ExternalInput")
    m_d = nc.dram_tensor("MT", [P, DT, DT, P], BF16, kind="ExternalInput")
    wv_d = nc.dram_tensor("WvT", [P, DT, E], BF16, kind="ExternalInput")
    msk_d = nc.dram_tensor("masks", [P, NSLOT, KCH], F32, kind="ExternalInput")
    out_d = nc.dram_tensor("out", [SQ, E], F32, kind="ExternalOutput")

    with tile.TileContext(nc) as tc, ExitStack() as ctx:
        # persistent tensors (right side)
        kqv = ctx.enter_context(tc.tile_pool(name="kqv", bufs=1, side="right"))
        xT = kqv.tile([P, DT, S], BF16, tag="xT")        # keys: x^T
        xn = kqv.tile([P, S // P, D], BF16, tag="xn")    # x natural [kb, d]
        qMT = kqv.tile([P, DT, SQ], BF16, tag="qMT")     # (xq M)^T
        wvT = kqv.tile([P, DT, E], BF16, tag="wvT")
        msk = kqv.tile([P, NSLOT, KCH], F32, tag="msk")

        # ---------------- folded q projection ----------------
        with (
            tc.tile_pool(name="wpool", bufs=1) as wpool,
            tc.tile_pool(name="xpool", bufs=3) as xpool,
            tc.tile_pool(name="pps", bufs=6, space="PSUM") as pps,
        ):
            # HAM warm-up: ~20 dummy matmuls on a zeroed tile fill the
            # DMA-init dead zone and un-throttle the PE clock before real work
            warm = xpool.tile([P, 512], BF16, tag="warm", name="warm", bufs=1)
            nc.gpsimd.memset(warm[:], 0.0)
            wps = pps.tile([P, 512], F32, tag="wps", name="wps", bufs=1)
            for _ in range(20):
                nc.tensor.matmul(
                    wps[:], lhsT=warm[:, 0:P], rhs=warm[:], start=True, stop=True
                )

            m_sb = wpool.tile([P, DT, DT, P], BF16, tag="M", name="m_sb")
            lo, hi = WPIECES[0]
            nc.sync.dma_start(m_sb[:, lo:hi], m_d[:, lo:hi])
            xqc = []
            t0 = 0
            for ci, csz in enumerate(QCH):
                xc = xpool.tile([P, DT, 512], BF16, tag="x", name="xc")
                nc.sync.dma_start(xc[:, :, 0:csz], xqT_d[:, :, t0 : t0 + csz])
                xqc.append(xc)
                t0 += csz
                if ci == 0:
                    for lo, hi in WPIECES[1:]:
                        nc.sync.dma_start(m_sb[:, lo:hi], m_d[:, lo:hi])
            # bulk streaming inputs, ordered by first use in the attention
            # phases: xT (QK), masks (first causal edge ~45us), xn (pass A),
            # WvT (pass B)
            nc.sync.dma_start(xT[:], xT_d[:])
            nc.sync.dma_start(msk[:], msk_d[:])
            nc.sync.dma_start(xn[:], xn_d[:])
            for lo, hi in WPIECES:
                nc.sync.dma_start(wvT[:, lo:hi], wv_d[:, lo:hi])

            t0 = 0
            for ci, csz in enumerate(QCH):
                xc = xqc[ci]
                for j_t in range(DT):
                    ps = pps.tile([P, 512], F32, tag="ps", name="ps")
                    for d in range(DT):
                        nc.tensor.matmul(
                            ps[:, 0:csz],
                            lhsT=m_sb[:, j_t, d, :],
                            rhs=xc[:, d, 0:csz],
                            start=(d == 0),
                            stop=(d == DT - 1),
                        )
                    nc.scalar.copy(qMT[:, j_t, t0 : t0 + csz], ps[:, 0:csz])
                t0 += csz

        # ---------------- attention ----------------
        with (
            tc.tile_pool(name="apool", bufs=2) as apool,
            tc.tile_pool(name="wtpool", bufs=4) as wtpool,
            tc.tile_pool(name="wxtpool", bufs=NSLOT) as wxtpool,
            tc.tile_pool(name="stpool", bufs=NSLOT, side="right") as stpool,
            tc.tile_pool(name="c1pool", bufs=1) as c1pool,
        ):
            ident = c1pool.tile([P, P], BF16, tag="ident")
            make_identity(nc, ident[:])

            def emit_scores(j):
                """QK (512-wide chunks) + mask + max + exp + sum for slot j."""
                C = CJ[j]
                L = C * KCH
                scores = apool.tile([P, S], F32, tag="scores", name="scores")
                groups = [(g * 512, 512) for g in range(C // 2)]
                if C % 2:
                    groups.append(((C // 2) * 512, 256))
                for k0, ksz in groups:
                    ps = qkps.tile([P, 512], F32, tag="qk", name="qk")
                    for d in range(DT):
                        nc.tensor.matmul(
                            ps[:, 0:ksz],
                            lhsT=qMT[:, d, j * P : (j + 1) * P],
                            rhs=xT[:, d, k0 : k0 + ksz],
                            start=(d == 0),
                            stop=(d == DT - 1),
                        )
                    last = k0 + ksz == L
                    if last and ksz == 512:
                        nc.vector.tensor_copy(
                            scores[:, k0 : k0 + 256], ps[:, 0:256]
                        )
                        nc.vector.tensor_add(
                            scores[:, k0 + 256 : k0 + 512],
                            ps[:, 256:512],
                            msk[:, j, :],
                        )
                    elif last:
                        nc.vector.tensor_add(
                            scores[:, k0 : k0 + 256], ps[:, 0:256], msk[:, j, :]
                        )
                    else:
                        nc.vector.tensor_copy(
                            scores[:, k0 : k0 + ksz], ps[:, 0:ksz]
                        )

                st = stpool.tile([P, 4], F32, tag="st", name="st")
                nc.vector.tensor_reduce(
                    st[:, 0:1], scores[:, 0:L], axis=AX, op=mybir.AluOpType.max
                )
                nc.scalar.mul(st[:, 1:2], st[:, 0:1], -INV_SQRT_DK)
                wts = apool.tile(
                    [P, S], BF16, tag="wts", name="wts", bufs=NSLOT
                )
                nc.scalar.activation(
                    wts[:, 0:L],
                    scores[:, 0:L],
                    EXP,
                    bias=st[:, 1:2],
                    scale=INV_SQRT_DK,
                    accum_out=st[:, 2:3],
                )
                nc.vector.reciprocal(st[:, 3:4], st[:, 2:3])
                return wts, st

            with tc.tile_pool(name="qkps", bufs=4, space="PSUM") as qkps:
                staged = [(j, *emit_scores(j)) for j in SLOT_ORDER]

            # ---- PV pass A: w transposes + (w @ x); previous slot's (wx)
            # transposes interleave so PE doesn't wait on ACT copies.
            wxT_all = []

            with (
                tc.tile_pool(name="wxps", bufs=4, space="PSUM") as wxps,
                tc.tile_pool(name="trps", bufs=4, space="PSUM") as trps,
            ):

                def make_wx_tr(si):
                    """per-d emitters: transpose (wx)[q,d] -> wxT[d,q]."""
                    wx_sb, _ = wx_staged[si]
                    wxT = wxtpool.tile(
                        [P, DT, P], BF16, tag="wxT", name="wxT"
                    )
                    wxT_all.append(wxT)

                    def emit_one(d):
                        pt = trps.tile([P, P], BF16, tag="tr", name="pt")
                        nc.tensor.transpose(
                            pt[:], wx_sb[:, d * P : (d + 1) * P], ident[:]
                        )
                        nc.vector.tensor_copy(wxT[:, d, :], pt[:])

                    return [emit_one(d) if False else (lambda d=d: emit_one(d))
                            for d in range(DT)]

                wx_staged = []
                pending_tr = []
                for si, (j, wts, st) in enumerate(staged):
                    nkb = CJ[j] * KCH // P
                    # weight transposes (one block lookahead inside the slot)
                    wTq = []

                    def emit_tr(kb, wts=wts):
                        pt = trps.tile([P, P], BF16, tag="tr", name="pt")
                        nc.tensor.transpose(
                            pt[:], wts[:, kb * P : (kb + 1) * P], ident[:]
                        )
                        wT = wtpool.tile([P, P], BF16, tag="wT", name="wT")
                        nc.vector.tensor_copy(wT[:], pt[:])
                        wTq.append(wT)

                    emit_tr(0)
                    if nkb > 1:
                        emit_tr(1)
                    po = [
                        wxps.tile([P, 512], F32, tag="wx", name=f"wx{ec}")
                        for ec in range(2)
                    ]
                    for kb in range(nkb):
                        if kb + 2 < nkb:
                            emit_tr(kb + 2)
                        if pending_tr:
                            pending_tr.pop(0)()
                        for ec in range(2):
                            nc.tensor.matmul(
                                po[ec][:],
                                lhsT=wTq[kb][:],
                                rhs=xn[:, kb, ec * 512 : (ec + 1) * 512],
                                start=(kb == 0),
                                stop=(kb == nkb - 1),
                            )
                    wx_sb = apool.tile(
                        [P, E], BF16, tag="wx", name="wx_sb", bufs=3
                    )
                    for ec in range(2):
                        nc.scalar.copy(
                            wx_sb[:, ec * 512 : (ec + 1) * 512], po[ec][:]
                        )
                    wx_staged.append((wx_sb, st))
                    for fn in pending_tr:
                        fn()
                    pending_tr = make_wx_tr(si)
                for fn in pending_tr:
                    fn()

            # ---- PV pass B: (wx)^T @ Wv^T, scaled by 1/sum, DMA out.
            with tc.tile_pool(name="pvps", bufs=4, space="PSUM") as pvps:
                for si, (j, _, st) in enumerate(staged):
                    wxT = wxT_all[si]
                    po = [
                        pvps.tile([P, 512], F32, tag="pv", name=f"po{ec}")
                        for ec in range(2)
                    ]
                    for d in range(DT):
                        for ec in range(2):
                            nc.tensor.matmul(
                                po[ec][:],
                                lhsT=wxT[:, d, :],
                                rhs=wvT[:, d, ec * 512 : (ec + 1) * 512],
                                start=(d == 0),
                                stop=(d == DT - 1),
                            )
                    ot = apool.tile([P, E], F32, tag="out", name="ot")
                    nc.scalar.mul(ot[:, 0:512], po[0][:], st[:, 3:4])
                    nc.sync.dma_start(
                        out_d[j * P : (j + 1) * P, 0:512], ot[:, 0:512]
                    )
                    nc.vector.tensor_scalar_mul(
                        ot[:, 512:1024], po[1][:], st[:, 3:4]
                    )
                    nc.sync.dma_start(
                        out_d[j * P : (j + 1) * P, 512:1024], ot[:, 512:1024]
                    )

    nc.compile()
    return nc


_NC_CACHE = None


def _get_nc():
    global _NC_CACHE
    if _NC_CACHE is None:
        _NC_CACHE = build_kernel()
    return _NC_CACHE


def _pack_inputs(x, Wq, Wk, Wv):
    """Host-side relayout + weight folding."""
    bf = ml_dtypes.bfloat16

    # folded scores matrix: scores = xq @ M @ xk^T with M = Wq^T @ Wk.
    # packed like a torch-Linear weight W_eff = M^T, lhsT[i,j] slices:
    # [p, j_t, i_t, j_local] = M[i_t*128+p, j_t*128+j_local]
    Mt = (Wk.T.astype(np.float64) @ Wq.astype(np.float64)).astype(np.float32)
    mp = np.ascontiguousarray(
        Mt.reshape(DT, P, DT, P).transpose(3, 0, 2, 1).astype(bf)
    )
    # Wv packed d-outer: [p, d, e] = Wv[e, d*128+p] (contiguous rhs slices)
    wvp = np.ascontiguousarray(
        Wv.reshape(E, DT, P).transpose(2, 1, 0).astype(bf)
    )

    # causal masks per slot (identical formula for both cores' block lists)
    def packmask(blocks):
        m = np.zeros((NSLOT, P, KCH), np.float32)
        for j, blk in enumerate(blocks):
            cc = np.arange(KCH)[None, :] + (CJ[j] - 1) * KCH  # key col
            rr = np.arange(P)[:, None] + blk * P              # query row
            m[j] = np.where(cc <= rr, 0.0, MASK_VAL)
        return np.ascontiguousarray(m.transpose(1, 0, 2))     # [P, slot, KCH]

    masks = [packmask(QBLOCKS[0]), packmask(QBLOCKS[1])]

    in_maps = []
    for c in range(N_CORES):
        b, h = divmod(c, 2)
        xb = x[b]  # [S, D]
        xt = np.ascontiguousarray(
            xb.reshape(S, DT, P).transpose(2, 1, 0).astype(bf)
        )
        xnat = np.ascontiguousarray(
            xb.reshape(S // P, P, D).transpose(1, 0, 2).astype(bf)
        )
        rows = np.concatenate(
            [np.arange(blk * P, (blk + 1) * P) for blk in QBLOCKS[h]]
        )
        xq = xb[rows]  # [SQ, D]
        xqt = np.ascontiguousarray(
            xq.reshape(SQ, DT, P).transpose(2, 1, 0).astype(bf)
        )
        in_maps.append(
            {
                "xT": xt,
                "xn": xnat,
                "xqT": xqt,
                "MT": mp,
                "WvT": wvp,
                "masks": masks[h],
            }
        )
    return in_maps


def kernel(x, Wq, Wk, Wv, _spmd_kwargs=None, _results_out=None):
    x = np.asarray(x, dtype=np.float32)
    Wq = np.asarray(Wq, dtype=np.float32)
    Wk = np.asarray(Wk, dtype=np.float32)
    Wv = np.asarray(Wv, dtype=np.float32)
    assert x.shape == (B, S, D)

    nc = _get_nc()
    in_maps = _pack_inputs(x, Wq, Wk, Wv)
    res = run_bass_kernel_spmd(
        nc, in_maps, list(range(N_CORES)), **(_spmd_kwargs or {})
    )
    if _results_out is not None:
        _results_out.append(res)

    out = np.empty((B, S, E), np.float32)
    for c in range(N_CORES):
        b, h = divmod(c, 2)
        o = res.results[c]["out"]
        for j, blk in enumerate(QBLOCKS[h]):
            out[b, blk * P : (blk + 1) * P, :] = o[j * P : (j + 1) * P, :]
    return out


# revision 17
# speedup vs baseline: 1.0197x; 1.0126x over previous
"""Trainium2 Bass kernel for single-head causal attention.

Problem: x[4,2048,1024] f32; Wq/Wk/Wv [1024,1024] (torch Linear layout, y = x@W.T).
  q,k,v = x@W.T ; scores = q@k.T (causal masked, scaled 1/sqrt(1024)) ;
  out = softmax(scores)@v.

Weight folding (the key algebraic move): scores = xq (Wq^T Wk) xk^T, so with
M := Wq^T Wk precomputed on the host, the K projection disappears entirely --
x^T itself is the key matrix. Likewise out = w @ x @ Wv^T, so the V projection
collapses to a small per-slot (w.x) @ Wv^T postmultiply. Device matmul work
drops from q/k/v projections + attention to: one folded q-projection
(xq @ M), QK against raw x^T, (w @ x), and (wx) @ Wv^T.

Sharding: 2 cores per batch (4 batches x 2 = 8 cores). Within a batch the 16
query blocks of 128 rows are split zig-zag so both cores get identical work
AND an identical program structure: core h=0 gets blocks [0,15,2,13,4,11,6,9],
h=1 gets [1,14,3,12,5,10,7,8]. Both orderings give causal key extents of
[1,8,2,7,3,6,4,5] chunks of 256 keys per slot, so a single SPMD program serves
all 8 cores; per-core data (x slices/transposes, gathered query rows, causal
masks, folded M) is prepared on the host.

Per-core pipeline (bf16 matmul inputs, fp32 PSUM accumulation):
  1. qMT = (xq @ M)^T via PE (the only projection, 1024 rows).
  2. QK phase (slots in descending causal length): scores chunks vs resident
     x^T, host mask on the causal edge, row-max -> exp((s-max)/32) on ACT with
     accumulated row sum; all softmax chains hide under later slots' matmuls.
  3. PV pass A: per slot, PE-transpose of weight blocks + (w @ x) accumulation
     over key blocks, with the (wx) transposes of the previous slot
     interleaved so PE never waits on ACT copies.
  4. PV pass B: per slot, (wx)^T @ Wv^T accumulated over d, 1/sum scaling
     fused into the PSUM->SBUF out copy, DMA out.
"""

from contextlib import ExitStack

import ml_dtypes
import numpy as np

import concourse.mybir as mybir
import concourse.tile as tile
from concourse import bacc
from concourse.bass_utils import run_bass_kernel_spmd
from concourse.masks import make_identity

B, S, D, E = 4, 2048, 1024, 1024
P = 128
N_CORES = 8
DT = D // P          # 8 d-tiles (contraction)
SQ = S // 2          # 1024 query rows per core
KCH = 256            # causal-length granularity (key chunk)
NSLOT = SQ // P      # 8 query slots per core

QCH = [256, 256, 512]             # xqT chunking (small first for startup)
assert sum(QCH) == SQ

# zig-zag query-block assignment: both cores' slots have identical causal
# chunk counts CJ, so one SPMD program serves all cores.
QBLOCKS = [[0, 15, 2, 13, 4, 11, 6, 9], [1, 14, 3, 12, 5, 10, 7, 8]]
CJ = [(b + 1 + 1) // 2 for b in QBLOCKS[0]]  # [1,8,2,7,3,6,4,5]
assert CJ == [(b + 1 + 1) // 2 for b in QBLOCKS[1]]
SLOT_ORDER = sorted(range(NSLOT), key=lambda j: -CJ[j])  # longest first

F32 = mybir.dt.float32
BF16 = mybir.dt.bfloat16
AX = mybir.AxisListType.X
EXP = mybir.ActivationFunctionType.Exp
INV_SQRT_DK = 1.0 / 32.0
MASK_VAL = -1.0e9
WPIECES = [(0, 1), (1, 2), (2, 4), (4, 8)]  # M DMA split over out-tiles


def build_kernel():
    nc = bacc.Bacc(
        "TRN2",
        target_bir_lowering=False,
        debug=False,
        num_devices=N_CORES,
        dynamic_dma_scratch_size=64,
    )
    xT_d = nc.dram_tensor("xT", [P, DT, S], BF16, kind="ExternalInput")
    xn_d = nc.dram_tensor("xn", [P, S // P, D], BF16, kind="ExternalInput")
    xqT_d = nc.dram_tensor("xqT", [P, DT, SQ], BF16, kind="ExternalInput")
    m_d = nc.dram_tensor("MT", [P, DT, DT, P], BF16, kind="ExternalInput")
    wv_d = nc.dram_tensor("WvT", [P, DT, E], BF16, kind="ExternalInput")
    msk_d = nc.dram_tensor("masks", [P, NSLOT, KCH], F32, kind="ExternalInput")
    out_d = nc.dram_tensor("out", [SQ, E], F32, kind="ExternalOutput")

    with tile.TileContext(nc) as tc, ExitStack() as ctx:
        # persistent tensors (right side)
        kqv = ctx.enter_context(tc.tile_pool(name="kqv", bufs=1, side="right"))
        xT = kqv.tile([P, DT, S], BF16, tag="xT")        # keys: x^T
        xn = kqv.tile([P, S // P, D], BF16, tag="xn")    # x natural [kb, d]
        qMT = kqv.tile([P, DT, SQ], BF16, tag="qMT")     # (xq M)^T
        wvT = kqv.tile([P, DT, E], BF16, tag="wvT")
        msk = kqv.tile([P, NSLOT, KCH], F32, tag="msk")

        # ---------------- folded q projection ----------------
        with (
            tc.tile_pool(name="wpool", bufs=1) as wpool,
            tc.tile_pool(name="xpool", bufs=3) as xpool,
            tc.tile_pool(name="pps", bufs=6, space="PSUM") as pps,
        ):
            # HAM warm-up: ~20 dummy matmuls on a zeroed tile fill the
            # DMA-init dead zone and un-throttle the PE clock before real work
            warm = xpool.tile([P, 512], BF16, tag="warm", name="warm", bufs=1)
            nc.gpsimd.memset(warm[:], 0.0)
            wps = pps.tile([P, 512], F32, tag="wps", name="wps", bufs=1)
            for _ in range(22):
                nc.tensor.matmul(
                    wps[:], lhsT=warm[:, 0:P], rhs=warm[:], start=True, stop=True
                )
            for _ in range(8):
                nc.tensor.matmul(
                    wps[:, 0:256],
                    lhsT=warm[:, 0:P],
                    rhs=warm[:, 0:256],
                    start=True,
                    stop=True,
                )

            m_sb = wpool.tile([P, DT, DT, P], BF16, tag="M", name="m_sb")
            lo, hi = WPIECES[0]
            nc.sync.dma_start(m_sb[:, lo:hi], m_d[:, lo:hi])
            xqc = []
            t0 = 0
            for ci, csz in enumerate(QCH):
                xc = xpool.tile([P, DT, 512], BF16, tag="x", name="xc")
                nc.sync.dma_start(xc[:, :, 0:csz], xqT_d[:, :, t0 : t0 + csz])
                xqc.append(xc)
                t0 += csz
                if ci == 0:
                    for lo, hi in WPIECES[1:]:
                        nc.sync.dma_start(m_sb[:, lo:hi], m_d[:, lo:hi])
            # bulk streaming inputs, ordered by first use in the attention
            # phases: xT (QK), masks (first causal edge ~45us), xn (pass A),
            # WvT (pass B)
            nc.sync.dma_start(xT[:], xT_d[:])
            nc.sync.dma_start(msk[:], msk_d[:])
            nc.sync.dma_start(xn[:], xn_d[:])
            for lo, hi in WPIECES:
                nc.sync.dma_start(wvT[:, lo:hi], wv_d[:, lo:hi])

            t0 = 0
            for ci, csz in enumerate(QCH):
                xc = xqc[ci]
                for j_t in range(DT):
                    ps = pps.tile([P, 512], F32, tag="ps", name="ps")
                    for d in range(DT):
                        nc.tensor.matmul(
                            ps[:, 0:csz],
                            lhsT=m_sb[:, j_t, d, :],
                            rhs=xc[:, d, 0:csz],
                            start=(d == 0),
                            stop=(d == DT - 1),
                        )
                    nc.scalar.copy(qMT[:, j_t, t0 : t0 + csz], ps[:, 0:csz])
                t0 += csz

        # ---------------- attention ----------------
        with (
            tc.tile_pool(name="apool", bufs=2) as apool,
            tc.tile_pool(name="wtpool", bufs=4) as wtpool,
            tc.tile_pool(name="wxtpool", bufs=NSLOT) as wxtpool,
            tc.tile_pool(name="stpool", bufs=NSLOT, side="right") as stpool,
            tc.tile_pool(name="c1pool", bufs=1) as c1pool,
        ):
            ident = c1pool.tile([P, P], BF16, tag="ident")
            make_identity(nc, ident[:])

            def emit_scores(j):
                """QK (512-wide chunks) + mask + max + exp + sum for slot j."""
                C = CJ[j]
                L = C * KCH
                scores = apool.tile([P, S], F32, tag="scores", name="scores")
                groups = [(g * 512, 512) for g in range(C // 2)]
                if C % 2:
                    groups.append(((C // 2) * 512, 256))
                for k0, ksz in groups:
                    ps = qkps.tile([P, 512], F32, tag="qk", name="qk")
                    for d in range(DT):
                        nc.tensor.matmul(
                            ps[:, 0:ksz],
                            lhsT=qMT[:, d, j * P : (j + 1) * P],
                            rhs=xT[:, d, k0 : k0 + ksz],
                            start=(d == 0),
                            stop=(d == DT - 1),
                        )
                    last = k0 + ksz == L
                    if last and ksz == 512:
                        nc.vector.tensor_copy(
                            scores[:, k0 : k0 + 256], ps[:, 0:256]
                        )
                        nc.vector.tensor_add(
                            scores[:, k0 + 256 : k0 + 512],
                            ps[:, 256:512],
                            msk[:, j, :],
                        )
                    elif last:
                        nc.vector.tensor_add(
                            scores[:, k0 : k0 + 256], ps[:, 0:256], msk[:, j, :]
                        )
                    else:
                        nc.vector.tensor_copy(
                            scores[:, k0 : k0 + ksz], ps[:, 0:ksz]
                        )

                st = stpool.tile([P, 4], F32, tag="st", name="st")
                nc.vector.tensor_reduce(
                    st[:, 0:1], scores[:, 0:L], axis=AX, op=mybir.AluOpType.max
                )
                nc.scalar.mul(st[:, 1:2], st[:, 0:1], -INV_SQRT_DK)
                wts = apool.tile(
                    [P, S], BF16, tag="wts", name="wts", bufs=NSLOT
                )
                nc.scalar.activation(
                    wts[:, 0:L],
                    scores[:, 0:L],
                    EXP,
                    bias=st[:, 1:2],
                    scale=INV_SQRT_DK,
                    accum_out=st[:, 2:3],
                )
                nc.vector.reciprocal(st[:, 3:4], st[:, 2:3])
                return wts, st

            with tc.tile_pool(name="qkps", bufs=4, space="PSUM") as qkps:
                staged = [(j, *emit_scores(j)) for j in SLOT_ORDER]

            # ---- PV pass A: w transposes + (w @ x); previous slot's (wx)
            # transposes interleave so PE doesn't wait on ACT copies.
            wxT_all = []

            with (
                tc.tile_pool(name="wxps", bufs=4, space="PSUM") as wxps,
                tc.tile_pool(name="trps", bufs=4, space="PSUM") as trps,
            ):

                def make_wx_tr(si):
                    """per-d emitters: transpose (wx)[q,d] -> wxT[d,q]."""
                    wx_sb, _ = wx_staged[si]
                    wxT = wxtpool.tile(
                        [P, DT, P], BF16, tag="wxT", name="wxT"
                    )
                    wxT_all.append(wxT)

                    def emit_one(d):
                        pt = trps.tile([P, P], BF16, tag="tr", name="pt")
                        nc.tensor.transpose(
                            pt[:], wx_sb[:, d * P : (d + 1) * P], ident[:]
                        )
                        nc.vector.tensor_copy(wxT[:, d, :], pt[:])

                    return [emit_one(d) if False else (lambda d=d: emit_one(d))
                            for d in range(DT)]

                wx_staged = []
                pending_tr = []
                for si, (j, wts, st) in enumerate(staged):
                    nkb = CJ[j] * KCH // P
                    # weight transposes (one block lookahead inside the slot)
                    wTq = []

                    def emit_tr(kb, wts=wts):
                        pt = trps.tile([P, P], BF16, tag="tr", name="pt")
                        nc.tensor.transpose(
                            pt[:], wts[:, kb * P : (kb + 1) * P], ident[:]
                        )
                        wT = wtpool.tile([P, P], BF16, tag="wT", name="wT")
                        nc.vector.tensor_copy(wT[:], pt[:])
                        wTq.append(wT)

                    emit_tr(0)
                    if nkb > 1:
                        emit_tr(1)
                    po = [
                        wxps.tile([P, 512], F32, tag="wx", name=f"wx{ec}")
                        for ec in range(2)
                    ]
                    for kb in range(nkb):
                        if kb + 2 < nkb:
                            emit_tr(kb + 2)
                        if pending_tr:
                            pending_tr.pop(0)()
                        for ec in range(2):
                            nc.tensor.matmul(
                                po[ec][:],
                                lhsT=wTq[kb][:],
                                rhs=xn[:, kb, ec * 512 : (ec + 1) * 512],
                                start=(kb == 0),
                                stop=(kb == nkb - 1),
                            )
                    wx_sb = apool.tile(
                        [P, E], BF16, tag="wx", name="wx_sb", bufs=3
                    )
                    for ec in range(2):
                        nc.scalar.copy(
                            wx_sb[:, ec * 512 : (ec + 1) * 512], po[ec][:]
                        )
                    wx_staged.append((wx_sb, st))
                    for fn in pending_tr:
                        fn()
                    pending_tr = make_wx_tr(si)
                for fn in pending_tr:
                    fn()

            # ---- PV pass B: (wx)^T @ Wv^T, scaled by 1/sum, DMA out.
            with tc.tile_pool(name="pvps", bufs=4, space="PSUM") as pvps:
                for si, (j, _, st) in enumerate(staged):
                    wxT = wxT_all[si]
                    po = [
                        pvps.tile([P, 512], F32, tag="pv", name=f"po{ec}")
                        for ec in range(2)
                    ]
                    for d in range(DT):
                        for ec in range(2):
                            nc.tensor.matmul(
                                po[ec][:],
                                lhsT=wxT[:, d, :],
                                rhs=wvT[:, d, ec * 512 : (ec + 1) * 512],
                                start=(d == 0),
                                stop=(d == DT - 1),
                            )
                    ot = apool.tile([P, E], F32, tag="out", name="ot")
                    nc.scalar.mul(ot[:, 0:512], po[0][:], st[:, 3:4])
                    nc.sync.dma_start(
                        out_d[j * P : (j + 1) * P, 0:512], ot[:, 0:512]
                    )
                    nc.vector.tensor_scalar_mul(
                        ot[:, 512:1024], po[1][:], st[:, 3:4]
                    )
                    nc.sync.dma_start(
                        out_d[j * P : (j + 1) * P, 512:1024], ot[:, 512:1024]
                    )

    nc.compile()
    return nc


_NC_CACHE = None


def _get_nc():
    global _NC_CACHE
    if _NC_CACHE is None:
        _NC_CACHE = build_kernel()
    return _NC_CACHE


def _pack_inputs(x, Wq, Wk, Wv):
    """Host-side relayout + weight folding."""
    bf = ml_dtypes.bfloat16

    # folded scores matrix: scores = xq @ M @ xk^T with M = Wq^T @ Wk.
    # packed like a torch-Linear weight W_eff = M^T, lhsT[i,j] slices:
    # [p, j_t, i_t, j_local] = M[i_t*128+p, j_t*128+j_local]
    Mt = (Wk.T.astype(np.float64) @ Wq.astype(np.float64)).astype(np.float32)
    mp = np.ascontiguousarray(
        Mt.reshape(DT, P, DT, P).transpose(3, 0, 2, 1).astype(bf)
    )
    # Wv packed d-outer: [p, d, e] = Wv[e, d*128+p] (contiguous rhs slices)
    wvp = np.ascontiguousarray(
        Wv.reshape(E, DT, P).transpose(2, 1, 0).astype(bf)
    )

    # causal masks per slot (identical formula for both cores' block lists)
    def packmask(blocks):
        m = np.zeros((NSLOT, P, KCH), np.float32)
        for j, blk in enumerate(blocks):
            cc = np.arange(KCH)[None, :] + (CJ[j] - 1) * KCH  # key col
            rr = np.arange(P)[:, None] + blk * P              # query row
            m[j] = np.where(cc <= rr, 0.0, MASK_VAL)
        return np.ascontiguousarray(m.transpose(1, 0, 2))     # [P, slot, KCH]

    masks = [packmask(QBLOCKS[0]), packmask(QBLOCKS[1])]

    in_maps = []
    for c in range(N_CORES):
        b, h = divmod(c, 2)
        xb = x[b]  # [S, D]
        xt = np.ascontiguousarray(
            xb.reshape(S, DT, P).transpose(2, 1, 0).astype(bf)
        )
        xnat = np.ascontiguousarray(
            xb.reshape(S // P, P, D).transpose(1, 0, 2).astype(bf)
        )
        rows = np.concatenate(
            [np.arange(blk * P, (blk + 1) * P) for blk in QBLOCKS[h]]
        )
        xq = xb[rows]  # [SQ, D]
        xqt = np.ascontiguousarray(
            xq.reshape(SQ, DT, P).transpose(2, 1, 0).astype(bf)
        )
        in_maps.append(
            {
                "xT": xt,
                "xn": xnat,
                "xqT": xqt,
                "MT": mp,
                "WvT": wvp,
                "masks": masks[h],
            }
        )
    return in_maps


def kernel(x, Wq, Wk, Wv, _spmd_kwargs=None, _results_out=None):
    x = np.asarray(x, dtype=np.float32)
    Wq = np.asarray(Wq, dtype=np.float32)
    Wk = np.asarray(Wk, dtype=np.float32)
    Wv = np.asarray(Wv, dtype=np.float32)
    assert x.shape == (B, S, D)

    nc = _get_nc()
    in_maps = _pack_inputs(x, Wq, Wk, Wv)
    res = run_bass_kernel_spmd(
        nc, in_maps, list(range(N_CORES)), **(_spmd_kwargs or {})
    )
    if _results_out is not None:
        _results_out.append(res)

    out = np.empty((B, S, E), np.float32)
    for c in range(N_CORES):
        b, h = divmod(c, 2)
        o = res.results[c]["out"]
        for j, blk in enumerate(QBLOCKS[h]):
            out[b, blk * P : (blk + 1) * P, :] = o[j * P : (j + 1) * P, :]
    return out
